# revision 1
# baseline (speedup 1.0000x reference)
"""Davies-Bouldin index (segment_reduce) Trainium2 kernel.

Strategy (one pass over the data instead of the reference's two):
  segsum(|x - A_c|^2)[k] = Q_k - 2*A_k.S_k + n_k*|A_k|^2
with S_k = segsum(x), Q_k = segsum(|x|^2), n_k = counts. The device computes
only S and the per-dim square sums S2 (Q = S2.sum(-1)) via a one-hot matmul
over bf16 data, data-parallel over 8 NeuronCores; counts and all K-sized
math run on the host in fp64.

Device per 128-point block b of a supertile:
  H_b[p,k]  = (cls[p,b] == k)        (DVE tensor_scalar is_equal vs iota row)
  psum     += H_b^T @ [X_b | X_b^2]  (PE, fp32 PSUM accumulation)
X^2 is computed by the ACT engine (Square). Per-core output is [128, 128]
fp32: cols 0:64 = S rows, cols 64:128 = S2 rows; rows 100..127 are padding.
"""

from contextlib import ExitStack

import numpy as np
import ml_dtypes

# ---- hardcoded problem geometry (nn_DBI_44985487458968) ----
N_TOTAL = 2_000_000
D = 64
K = 100
N_CORES = 8
P = 128
KPAD = 128            # (legacy) padded one-hot width
KP = 100              # one-hot width = K (no FWL in this toolchain, no pad)
B = 32                # 128-point blocks per supertile
SUP = P * B           # 4096 points per supertile
PER_CORE = N_TOTAL // N_CORES          # 250_000
NSUP = -(-PER_CORE // SUP)             # 62 supertiles
PADN = NSUP * SUP                      # 253_952 padded points per core
PAD_CLS = 127          # pad points land in ignored one-hot column 127

BF16 = ml_dtypes.bfloat16


def _split_excess_waits(nc):
    """Walrus allows one semaphore wait per instruction (two on
    EventSemaphore). Tile's tail drain aggregates one wait per live proc,
    which this compiler build rejects — hoist the extras into standalone
    NoOp wait-carriers executed just before, same engine, same semantics."""
    import concourse.mybir as mybir

    for bb in nc.main_func.blocks:
        new = []
        for inst in bb.instructions:
            si = inst.sync_info
            limit = 2 if isinstance(inst, mybir.InstEventSemaphore) else 1
            if si is not None and si.on_wait and len(si.on_wait) > limit:
                waits = list(si.on_wait)
                for w in waits[:-limit]:
                    nop = mybir.InstNoOp(
                        name=nc.get_next_instruction_name(),
                        engine=inst.engine,
                        ins=[], outs=[],
                        sync_info=mybir.SyncInfo(on_wait=[w], on_update=[]),
                    )
                    nc.register_instruction(nop)
                    new.append(nop)
                inst.sync_info = mybir.SyncInfo(
                    on_wait=waits[-limit:], on_update=list(si.on_update))
            new.append(inst)
        bb.instructions[:] = new


def _build_module(nsup: int, b: int):
    import concourse.bass as bass
    import concourse.mybir as mybir
    import concourse.tile as tile

    sup_cols = b * D                      # X columns per supertile
    nc = bass.Bass()
    x_in = nc.dram_tensor("x", [nsup, P, sup_cols], mybir.dt.bfloat16,
                          kind="ExternalInput")
    cls_in = nc.dram_tensor("cls", [P, nsup * b], mybir.dt.bfloat16,
                            kind="ExternalInput")
    # iota_kb[p, k*b + j] = k, so one tensor_tensor is_equal against a
    # broadcast cls slice emits the whole supertile's one-hot at 2x mode.
    iota_in = nc.dram_tensor("iota", [P, KP * b], mybir.dt.bfloat16,
                             kind="ExternalInput")
    out = nc.dram_tensor("out", [KP, 2 * D], mybir.dt.float32,
                         kind="ExternalOutput")

    n_mm_total = nsup * b
    with ExitStack() as ctx:
        tc = ctx.enter_context(tile.TileContext(nc))
        cpool = ctx.enter_context(tc.tile_pool(name="const", bufs=1))
        xpool = ctx.enter_context(tc.tile_pool(name="x", bufs=3))
        hpool = ctx.enter_context(tc.tile_pool(name="h", bufs=3))
        ppool = ctx.enter_context(tc.tile_pool(name="psum", bufs=1, space="PSUM"))
        opool = ctx.enter_context(tc.tile_pool(name="o", bufs=1))

        iota_t = cpool.tile([P, KP * b], mybir.dt.bfloat16)
        nc.sync.dma_start(out=iota_t[:], in_=iota_in[:])
        cls_t = cpool.tile([P, nsup * b], mybir.dt.bfloat16)
        nc.sync.dma_start(out=cls_t[:], in_=cls_in[:])

        psum_t = ppool.tile([P, 2 * D], mybir.dt.float32)

        n_mm = 0
        for s in range(nsup):
            xb = xpool.tile([P, 2 * sup_cols], mybir.dt.bfloat16)
            nc.sync.dma_start(out=xb[:, 0:sup_cols], in_=x_in[s])
            nc.scalar.activation(
                out=xb[:, sup_cols:2 * sup_cols],
                in_=xb[:, 0:sup_cols],
                func=mybir.ActivationFunctionType.Square,
            )
            # One-hot for the whole supertile in one DVE op (2x mode):
            # G[p, k*b + j] = (cls[p, s*b+j] == k).
            gt = hpool.tile([P, KP * b], mybir.dt.bfloat16)
            cls_bc = cls_t[:, s * b:(s + 1) * b].unsqueeze(1).broadcast_to(
                [P, KP, b])
            nc.vector.tensor_tensor(
                out=gt[:], in0=iota_t[:], in1=cls_bc,
                op=mybir.AluOpType.is_equal,
            )
            gv = gt[:].rearrange("p (k j) -> p k j", j=b)
            xr = xb[:].rearrange("p (two n) -> p two n", two=2)
            for j in range(b):
                nc.tensor.matmul(
                    psum_t[:KP, :],
                    lhsT=gv[:, :, j],
                    rhs=xr[:, :, j * D:(j + 1) * D],
                    start=(n_mm == 0),
                    stop=(n_mm == n_mm_total - 1),
                )
                n_mm += 1

        out_sb = opool.tile([KP, 2 * D], mybir.dt.float32)
        nc.vector.tensor_copy(out=out_sb[:], in_=psum_t[:KP, :])
        nc.sync.dma_start(out=out[:], in_=out_sb[:])
    _split_excess_waits(nc)
    return nc


def _prep_core_inputs(x_shard: np.ndarray, cls_shard: np.ndarray,
                      nsup: int, b: int) -> dict:
    """Pad + lay out one core's shard for the device kernel."""
    sup = P * b
    padn = nsup * sup
    npts = x_shard.shape[0]
    xb16 = np.zeros((padn, D), dtype=BF16)
    xb16[:npts] = x_shard.astype(BF16)
    clsf = np.full((padn,), PAD_CLS, dtype=BF16)
    clsf[:npts] = cls_shard.astype(BF16)
    # xb[s, p, j*D+d] = x[s*sup + p*b + j, d]
    x_dev = np.ascontiguousarray(xb16.reshape(nsup, P, b * D))
    # cls_t[p, s*b + j] = cls[s*sup + p*b + j]
    cls_dev = np.ascontiguousarray(
        clsf.reshape(nsup, P, b).transpose(1, 0, 2).reshape(P, nsup * b))
    # iota_kb[p, k*b + j] = k
    iota = np.ascontiguousarray(np.broadcast_to(
        np.repeat(np.arange(KP), b).astype(BF16)[None, :], (P, KP * b)))
    return {"x": x_dev, "cls": cls_dev, "iota": iota}


def _dbi_from_stats(S: np.ndarray, S2: np.ndarray, n: np.ndarray) -> np.float32:
    S = S.astype(np.float64)
    Q = S2.astype(np.float64).sum(-1)
    n = n.astype(np.float64)
    counts = 1.0 + n
    A = (0.001 + S) / counts[:, None]
    segsq = Q - 2.0 * (A * S).sum(-1) + n * (A * A).sum(-1)
    Si = np.sqrt((0.001 + segsq) / counts)
    diff = A[:, None, :] - A[None, :, :]
    sumsq = (diff * diff).sum(-1)
    eye = np.eye(K, dtype=bool)
    Mij = np.sqrt(np.where(eye, 1.0, sumsq))
    Rij = np.where(eye, 0.0, (Si[:, None] + Si[None, :]) / Mij)
    return np.float32(Rij.max(axis=1).sum() / K)


def kernel(data_points: np.ndarray, clustering: np.ndarray) -> np.ndarray:
    from concourse.bass_utils import run_bass_kernel_spmd

    x = np.asarray(data_points)
    cls = np.asarray(clustering)
    assert x.shape == (N_TOTAL, D), x.shape

    nc = _build_module(NSUP, B)
    in_maps = []
    for c in range(N_CORES):
        sl = slice(c * PER_CORE, (c + 1) * PER_CORE)
        in_maps.append(_prep_core_inputs(x[sl], cls[sl], NSUP, B))
    res = run_bass_kernel_spmd(nc, in_maps, core_ids=list(range(N_CORES)))

    S = np.zeros((K, D), np.float64)
    S2 = np.zeros((K, D), np.float64)
    for r in res.results:
        o = r["out"].astype(np.float64)
        S += o[:K, :D]
        S2 += o[:K, D:]
    assert KP >= K
    n = np.bincount(cls.astype(np.int64), minlength=K).astype(np.float64)
    return np.asarray(_dbi_from_stats(S, S2, n), dtype=np.float32)



# revision 2
# speedup vs baseline: 1.0643x; 1.0643x over previous
"""Davies-Bouldin index (segment_reduce) Trainium2 kernel, v4: lane-mapped.

Host sorts points by cluster and assigns each of the 256 virtual lanes
(128 partitions x 2 DoubleRow halves) to one LOCAL cluster, with lane
counts proportional to cluster size (per-lane padding ~1-3%). The
stationary operand is then a single [128, 2, 16] lane->cluster one-hot,
identical for every matmul: it is loaded into the PE array once (repeat
matmuls carry ldweights=False) and fp8 DoubleRow matmuls stream
back-to-back, each contracting all 256 lanes over FD=455 cols (7 point
slots per lane, 1792 points per matmul), accumulating S|Q per local
cluster into psum[:16, :455]. Supertiles ramp small->large->small so the
first matmul starts early and the tail drains fast. Host fp64 finish.
"""

from contextlib import ExitStack

import numpy as np
import ml_dtypes

# ---- hardcoded problem geometry (nn_DBI_44985487458968) ----
N_TOTAL = 2_000_000
D = 64
K = 100
N_CORES = 8
P = 128
PER_CORE = N_TOTAL // N_CORES          # 250_000

DCOL = D + 1          # 64 dims + q column
WCOL = 16             # one-hot width (max distinct clusters per shard)
VL = 2 * P            # virtual lanes (DoubleRow halves)
MMB = 7               # point slots per lane per matmul (7*65 = 455 psum cols)
FD = MMB * DCOL       # 455 psum cols per matmul; rhs streams 2*FD
SUPM = 16             # matmuls per steady-state supertile
RAMP = [2, 2, 4, 8]   # matmuls in the leading (small) supertiles

BF16 = ml_dtypes.bfloat16
FP8 = ml_dtypes.float8_e4m3


def _schedule(c_max: int) -> list[int]:
    """Per-supertile matmul counts covering >= c_max point slots/lane."""
    sched = list(RAMP)
    covered = sum(sched) * MMB
    rem = max(0, c_max - covered)
    n_full = rem // (SUPM * MMB)
    sched += [SUPM] * n_full
    rem -= n_full * SUPM * MMB
    if rem > 0:
        sched.append(-(-rem // MMB))
    return sched


def _split_excess_waits(nc):
    """Walrus allows one semaphore wait per instruction (two on
    EventSemaphore). Tile's tail drain aggregates one wait per live proc,
    which this compiler build rejects — hoist the extras into standalone
    NoOp wait-carriers executed just before, same engine, same semantics."""
    import concourse.mybir as mybir

    for bb in nc.main_func.blocks:
        new = []
        for inst in bb.instructions:
            si = inst.sync_info
            limit = 2 if isinstance(inst, mybir.InstEventSemaphore) else 1
            if si is not None and si.on_wait and len(si.on_wait) > limit:
                waits = list(si.on_wait)
                for w in waits[:-limit]:
                    nop = mybir.InstNoOp(
                        name=nc.get_next_instruction_name(),
                        engine=inst.engine,
                        ins=[], outs=[],
                        sync_info=mybir.SyncInfo(on_wait=[w], on_update=[]),
                    )
                    nc.register_instruction(nop)
                    new.append(nop)
                inst.sync_info = mybir.SyncInfo(
                    on_wait=waits[-limit:], on_update=list(si.on_update))
            new.append(inst)
        bb.instructions[:] = new


def _build_module(sched: list[int]):
    import concourse.bass as bass
    import concourse.mybir as mybir
    import concourse.tile as tile

    nmm = sum(sched)
    tot_cols = 2 * nmm * FD
    nc = bass.Bass()
    x_in = nc.dram_tensor("x", [P, tot_cols], mybir.dt.float8e4,
                          kind="ExternalInput")
    # lanecode[p, i] = local cluster of virtual lane (p, i); iota = 0..15 x2
    lane_in = nc.dram_tensor("lane", [P, 2], mybir.dt.bfloat16,
                             kind="ExternalInput")
    iota_in = nc.dram_tensor("iota", [P, 2 * WCOL], mybir.dt.bfloat16,
                             kind="ExternalInput")
    out = nc.dram_tensor("out", [2, WCOL, FD], mybir.dt.float32,
                         kind="ExternalOutput")

    with ExitStack() as ctx:
        tc = ctx.enter_context(tile.TileContext(nc))
        cpool = ctx.enter_context(tc.tile_pool(name="const", bufs=1))
        xpool = ctx.enter_context(tc.tile_pool(name="x", bufs=6))
        ppool = ctx.enter_context(tc.tile_pool(name="psum", bufs=1, space="PSUM"))
        opool = ctx.enter_context(tc.tile_pool(name="o", bufs=1))

        # first data chunk goes out before anything else on the DMA queue
        xts = []
        off = 0
        for s, w in enumerate(sched):
            cols = 2 * w * FD
            xt = xpool.tile([P, cols], mybir.dt.float8e4)
            nc.sync.dma_start(out=xt[:], in_=x_in[:, off:off + cols])
            xts.append(xt)
            off += cols
            if s == 0:
                iota_t = cpool.tile([P, 2 * WCOL], mybir.dt.bfloat16)
                nc.sync.dma_start(out=iota_t[:], in_=iota_in[:])
                lane_t = cpool.tile([P, 2], mybir.dt.bfloat16)
                nc.sync.dma_start(out=lane_t[:], in_=lane_in[:])

        # the single stationary one-hot: wt[p, i, k] = (lanecode[p,i] == k)
        wt = cpool.tile([P, 2 * WCOL], mybir.dt.float8e4)
        nc.vector.tensor_tensor(
            out=wt[:].rearrange("p (i k) -> p i k", k=WCOL),
            in0=iota_t[:].rearrange("p (i k) -> p i k", k=WCOL),
            in1=lane_t[:].unsqueeze(2).broadcast_to([P, 2, WCOL]),
            op=mybir.AluOpType.is_equal,
        )
        wt_v = wt[:].rearrange("p (i k) -> p i k", k=WCOL)

        psum_a = ppool.tile([P, FD], mybir.dt.float32)
        psum_b = ppool.tile([P, FD], mybir.dt.float32)

        # PE warmup: keep HAM busy while the first supertile DMA lands.
        warm_sb = cpool.tile([P, 128], mybir.dt.bfloat16)
        nc.any.memset(warm_sb, 0)
        warm_ps = ppool.tile([P, 128], mybir.dt.float32)
        for _ in range(10):
            nc.tensor.matmul(
                warm_ps[:WCOL, :],
                lhsT=warm_sb[:, :WCOL],
                rhs=warm_sb[:, :],
                start=True, stop=True,
            )

        gmid = nmm // 2
        out_sb = opool.tile([P, 2 * FD], mybir.dt.float32)
        g = 0
        for s, w in enumerate(sched):
            xt_v = xts[s][:].rearrange("p (i mf) -> p i mf", i=2)
            for m in range(w):
                ep, pt = (0, psum_a) if g < gmid else (1, psum_b)
                mm = nc.tensor.matmul(
                    pt[:WCOL, :],
                    lhsT=wt_v,
                    rhs=xt_v[:, :, m * FD:(m + 1) * FD],
                    start=(g == 0 or g == gmid),
                    stop=(g == gmid - 1 or g == nmm - 1),
                    perf_mode=mybir.MatmulPerfMode.DoubleRow,
                )
                if g > 0:
                    mm.ldweights = False
                g += 1
                if g == gmid:
                    # epoch A done: drain it while epoch B keeps streaming
                    nc.vector.tensor_copy(out=out_sb[:WCOL, :FD],
                                          in_=psum_a[:WCOL, :])
                    nc.sync.dma_start(out=out[0], in_=out_sb[:WCOL, :FD])

        nc.vector.tensor_copy(out=out_sb[:WCOL, FD:], in_=psum_b[:WCOL, :])
        nc.sync.dma_start(out=out[1], in_=out_sb[:WCOL, FD:])
    _split_excess_waits(nc)
    return nc


def _core_plan(cls_shard: np.ndarray):
    """Lane assignment for one shard: lanes per cluster ~ cluster size."""
    uq, counts = np.unique(cls_shard, return_counts=True)
    assert len(uq) <= WCOL, f"{len(uq)} local clusters > {WCOL}"
    lanes = np.maximum(1, (VL * counts) // counts.sum()).astype(np.int64)
    while lanes.sum() > VL:
        lanes[np.argmax(lanes)] -= 1
    while lanes.sum() < VL:
        j = int(np.argmax(counts / lanes))
        lanes[j] += 1
    c_pts = int((-(-counts // lanes)).max())    # slots per lane needed
    return uq, counts, lanes, c_pts


def _prep_core_inputs(x_srt, q_srt, counts, lanes, sched) -> dict:
    """Lay out one core's cluster-sorted shard lane-wise for the device."""
    c_pad = sum(sched) * MMB
    vl_sizes = []
    vl_cluster = []
    for l, (cnt, nl) in enumerate(zip(counts, lanes)):
        base, rem = divmod(int(cnt), int(nl))
        sizes = np.full(nl, base, np.int64)
        sizes[:rem] += 1
        vl_sizes.append(sizes)
        vl_cluster.append(np.full(nl, l, np.int64))
    vl_sizes = np.concatenate(vl_sizes)          # [VL]
    vl_cluster = np.concatenate(vl_cluster)      # [VL]
    assert len(vl_sizes) == VL and vl_sizes.max() <= c_pad
    src_starts = np.concatenate(([0], np.cumsum(vl_sizes)[:-1]))
    pos = np.repeat(np.arange(VL) * c_pad - src_starts, vl_sizes) \
        + np.arange(len(x_srt))
    dst = np.zeros((VL * c_pad, DCOL), dtype=FP8)
    dst[pos, :D] = x_srt.astype(FP8)
    dst[pos, D] = q_srt.astype(FP8)
    # per supertile s (w matmuls, slot range [t0, t0+w*MMB)):
    # cols (i, t-t0, c) flattened; vlane (p, i) = i*128 + p
    dv = dst.reshape(2, P, c_pad, DCOL)
    segs = []
    t0 = 0
    for w in sched:
        wsl = w * MMB
        seg = dv[:, :, t0:t0 + wsl, :].transpose(1, 0, 2, 3) \
            .reshape(P, 2 * wsl * DCOL)
        segs.append(seg)
        t0 += wsl
    x_dev = np.ascontiguousarray(np.concatenate(segs, axis=1))
    lane_dev = np.ascontiguousarray(
        vl_cluster.reshape(2, P).T.astype(BF16))          # [P, 2]
    iota = np.ascontiguousarray(np.broadcast_to(
        np.tile(np.arange(WCOL), 2).astype(BF16)[None, :], (P, 2 * WCOL)))
    return {"x": x_dev, "lane": lane_dev, "iota": iota}


def _fold_out(out_arr: np.ndarray) -> np.ndarray:
    """[2, WCOL, FD] device output -> [WCOL, DCOL] per-local-cluster S|Q."""
    return out_arr.astype(np.float64).reshape(2 * WCOL, MMB, DCOL).sum(1) \
        .reshape(2, WCOL, DCOL).sum(0)


def _dbi_from_stats(S: np.ndarray, Q: np.ndarray, n: np.ndarray) -> np.float32:
    S = S.astype(np.float64)
    Q = Q.astype(np.float64)
    n = n.astype(np.float64)
    counts = 1.0 + n
    A = (0.001 + S) / counts[:, None]
    segsq = Q - 2.0 * (A * S).sum(-1) + n * (A * A).sum(-1)
    Si = np.sqrt((0.001 + segsq) / counts)
    diff = A[:, None, :] - A[None, :, :]
    sumsq = (diff * diff).sum(-1)
    eye = np.eye(K, dtype=bool)
    Mij = np.sqrt(np.where(eye, 1.0, sumsq))
    Rij = np.where(eye, 0.0, (Si[:, None] + Si[None, :]) / Mij)
    return np.float32(Rij.max(axis=1).sum() / K)


def _plan_and_prep(x: np.ndarray, cls: np.ndarray):
    q = np.einsum("nd,nd->n", x, x, dtype=np.float32)
    order = np.argsort(cls, kind="stable")
    plans = []
    for c in range(N_CORES):
        o = order[c * PER_CORE:(c + 1) * PER_CORE]
        uq, counts, lanes, c_pts = _core_plan(cls[o])
        plans.append((o, uq, counts, lanes, c_pts))
    c_max = max(p[4] for p in plans)
    sched = _schedule(c_max)
    in_maps = []
    for (o, uq, counts, lanes, c_pts) in plans:
        in_maps.append(_prep_core_inputs(x[o], q[o], counts, lanes, sched))
    return plans, sched, in_maps


def kernel(data_points: np.ndarray, clustering: np.ndarray) -> np.ndarray:
    from concourse.bass_utils import run_bass_kernel_spmd

    x = np.asarray(data_points)
    cls = np.asarray(clustering).astype(np.int64)
    assert x.shape == (N_TOTAL, D), x.shape

    plans, sched, in_maps = _plan_and_prep(x, cls)
    nc = _build_module(sched)
    res = run_bass_kernel_spmd(nc, in_maps, core_ids=list(range(N_CORES)))

    S = np.zeros((K, D), np.float64)
    Q = np.zeros(K, np.float64)
    for r, (o, uq, counts, lanes, c_pts) in zip(res.results, plans):
        oarr = _fold_out(r["out"])
        S[uq] += oarr[:len(uq), :D]
        Q[uq] += oarr[:len(uq), D]
    n = np.bincount(cls, minlength=K).astype(np.float64)
    return np.asarray(_dbi_from_stats(S, Q, n), dtype=np.float32)
